# revision 4
# baseline (speedup 1.0000x reference)
"""Gaussian-splat blend kernel for 8 TRN2 NeuronCores — v2 (packed ln).

Math (per pixel p, gaussians sorted nearest-first):
  q_g(p)   = (x_p - mu2d_g)^T inv_g (x_p - mu2d_g)      quadratic in x
  a_g(p)   = w_g * exp(-q/2),  z = -q/2 + ln w
  out_c(p) = sum_g a_g * prod_{j>g}(1-a_j) * color_gc + prod_all(1-a_j)*bg_c

ScalarE (ACT) is the bottleneck: it runs 128 lanes x 1 col/cycle with no
dtype speedup, so cost = total COLUMNS.  v1 did 3 full transcendental
passes (exp, ln, exp) = 98.7us ACT busy.  v2 splits the gaussians by
per-gaussian max opacity over the pixel box:
  strong (top 64 by max_a): exact l = ln(1-a), PACKED 2-up -> the ln
    pass covers two 512-px blocks per 128-partition tile (half width).
  nonstrong (bottom 64, max_a <= ~0.37): l ~= -(a + a^2/2); the
    quadratic-truncation error gives rel-err ~8e-4 (gate 2e-2).  The
    polynomial is ONE idle-DVE op: p' = (a+2)*a = 2a+a^2, with the 1/2
    folded into -0.5 entries of the triangular matmul weights.
ACT passes: exp1 (1.0) + ln (0.5) + exp2 (1.0) = 2.5 vs 3.0, and each
superstep is one [128,2048] PSUM tile so exp1/exp2 are single wide
instructions (fewer fixed ~240ns/instr overheads).

Layout per 2048-px superstep: one PSUM tile z [128, 2048] f32:
  cols    0:512  piece A: strong slots, blocks b0 (rows 0:64) | b1 (64:128)
  cols  512:1024 piece B: strong, b2 | b3
  cols 1024:1536 piece C: nonstrong, b0 | b1
  cols 1536:2048 piece D: nonstrong, b2 | b3
mm1: 4 block-diagonal matmuls (18-row error-compensated bf16 coeffs at
strip rows 0:18 / 32:50 of the v1 f18p feature layout).
ACT exp1: a = exp(z) [128,2048] f32.  ACT ln: l = ln(1-a[:,0:1024]) bf16.
DVE: p' = (a_ns+2)*a_ns bf16.  mm2: 8 matmuls accumulate S onto z
(blockdiag strict-order tri matrices; -0.5 weights for p').  ACT exp2:
w = exp(z+S) bf16.  mm3: 4 matmuls -> z[0:12,0:512] (freed rows), DVE
copy -> SBUF, DMA out.  exp2/mm3/copy are skewed one superstep so ACT
fills the mm2 latency with the next exp1.  Host adds bg and unpacks.
"""

import numpy as np
import ml_dtypes

import concourse.bass as bass
import concourse.bacc as bacc
import concourse.mybir as mybir
import concourse.tile as tile
from concourse.bass_utils import run_bass_kernel_spmd

G = 128
B = 4
N = 65536
BN = B * N
NCORES = 8
PPC = BN // NCORES          # pixels per core = 32768
SUPPX = 2048                # pixels per superstep
NSUP = PPC // SUPPX         # 16
FW = 1024                   # f18p cols per superstep (2 blocks packed/col)

F32 = mybir.dt.float32
BF16 = mybir.dt.bfloat16
AFT = mybir.ActivationFunctionType
ALU = mybir.AluOpType
BF = ml_dtypes.bfloat16

PROFILE = False
LAST_EXEC_NS = None
LAST_RESULTS = None

_cached = None


def _patch_act_tables():
    """Force every activation onto the one table set that has BOTH Exp and
    Ln ("natural_log_exp_and_others") — otherwise the table-load pass
    alternates sets and burns ~1.3us per ACT_TABLE_LOAD, once per tile."""
    if getattr(bacc, "_act_tables_patched", False):
        return
    orig = bacc.get_activation_tables

    def only_nle(arch):
        tabs = orig(arch)
        return {
            name: (fns if name == "natural_log_exp_and_others" else set())
            for name, fns in tabs.items()
        }

    bacc.get_activation_tables = only_nle
    bacc._act_tables_patched = True


def _build():
    _patch_act_tables()
    nc = bacc.Bacc("TRN2", target_bir_lowering=False, debug=False,
                   num_devices=NCORES)
    f18p = nc.dram_tensor("f18p", [64, PPC // 2], BF16, kind="ExternalInput")
    w1s = nc.dram_tensor("w1s", [64, G], BF16, kind="ExternalInput")
    w1n = nc.dram_tensor("w1n", [64, G], BF16, kind="ExternalInput")
    tss = nc.dram_tensor("tss", [G, G], BF16, kind="ExternalInput")
    tsn = nc.dram_tensor("tsn", [G, G], BF16, kind="ExternalInput")
    tns = nc.dram_tensor("tns", [G, G], BF16, kind="ExternalInput")
    tnn = nc.dram_tensor("tnn", [G, G], BF16, kind="ExternalInput")
    w3s = nc.dram_tensor("w3s", [G, 6], BF16, kind="ExternalInput")
    w3n = nc.dram_tensor("w3n", [G, 6], BF16, kind="ExternalInput")
    out = nc.dram_tensor("out", [12, PPC // 4], F32, kind="ExternalOutput")

    with tile.TileContext(nc) as tc:
        with (
            tc.tile_pool(name="const", bufs=1) as constp,
            tc.tile_pool(name="featp", bufs=3) as featp,
            tc.tile_pool(name="zp", bufs=2, space="PSUM") as zp,
            tc.tile_pool(name="aa", bufs=2) as ap_,
            tc.tile_pool(name="lp", bufs=2) as lp,
            tc.tile_pool(name="pp", bufs=2) as pp,
            tc.tile_pool(name="wp", bufs=2) as wp,
            tc.tile_pool(name="obuf", bufs=4) as obufp,
        ):
            # dependency-free dummy activation: pulls the ~1.3us
            # ACT_TABLE_LOAD into the idle DMA-wait head
            dummy = constp.tile([1, 8], F32)
            nc.gpsimd.memset(dummy[:], 0.0)
            nc.scalar.activation(dummy[:], dummy[:], AFT.Exp)

            fbufs = [featp.tile([64, FW], BF16, tag="fbuf", name=f"fbuf{i}")
                     for i in range(NSUP)]
            # first feature block + mm1 weights on the sync queue, the rest
            # on gpsimd so descriptor-gens run in parallel
            nc.sync.dma_start(fbufs[0][:], f18p[:, bass.ts(0, FW)])
            w1s_t = constp.tile([64, G], BF16)
            nc.sync.dma_start(w1s_t[:], w1s[:])
            w1n_t = constp.tile([64, G], BF16)
            nc.sync.dma_start(w1n_t[:], w1n[:])
            tss_t = constp.tile([G, G], BF16)
            nc.gpsimd.dma_start(tss_t[:], tss[:])
            tsn_t = constp.tile([G, G], BF16)
            nc.gpsimd.dma_start(tsn_t[:], tsn[:])
            tns_t = constp.tile([G, G], BF16)
            nc.gpsimd.dma_start(tns_t[:], tns[:])
            tnn_t = constp.tile([G, G], BF16)
            nc.gpsimd.dma_start(tnn_t[:], tnn[:])
            w3s_t = constp.tile([G, 6], BF16)
            nc.gpsimd.dma_start(w3s_t[:], w3s[:])
            w3n_t = constp.tile([G, 6], BF16)
            nc.gpsimd.dma_start(w3n_t[:], w3n[:])

            zs = [None] * NSUP
            wst = [None] * NSUP
            for p in range(NSUP + 1):
                if p < NSUP:
                    if p + 1 < NSUP:
                        nc.gpsimd.dma_start(fbufs[p + 1][:],
                                            f18p[:, bass.ts(p + 1, FW)])
                    fb = fbufs[p]
                    z = zp.tile([G, 2048], F32)
                    zs[p] = z
                    nc.tensor.matmul(z[:, 0:512], w1s_t[:], fb[:, 0:512],
                                     start=True, stop=False)
                    nc.tensor.matmul(z[:, 512:1024], w1s_t[:], fb[:, 512:1024],
                                     start=True, stop=False)
                    nc.tensor.matmul(z[:, 1024:1536], w1n_t[:], fb[:, 0:512],
                                     start=True, stop=False)
                    nc.tensor.matmul(z[:, 1536:2048], w1n_t[:],
                                     fb[:, 512:1024], start=True, stop=False)
                    a = ap_.tile([G, 2048], F32)
                    nc.scalar.activation(a[:], z[:], AFT.Exp)
                    l = lp.tile([G, 1024], BF16)
                    nc.scalar.activation(l[:], a[:, 0:1024], AFT.Ln,
                                         bias=1.0, scale=-1.0)
                    pq = pp.tile([G, 1024], BF16)
                    nc.vector.scalar_tensor_tensor(
                        pq[:], a[:, 1024:2048], 2.0, a[:, 1024:2048],
                        ALU.add, ALU.mult)
                    for c0, mv in ((0, l), (1024, pq)):
                        mats = (tss_t, tsn_t) if c0 == 0 else (tns_t, tnn_t)
                        for piece in range(2):
                            src = mv[:, bass.ts(piece, 512)]
                            nc.tensor.matmul(
                                z[:, bass.ds(piece * 512, 512)], mats[0][:],
                                src, start=False, stop=(c0 == 1024))
                            nc.tensor.matmul(
                                z[:, bass.ds(1024 + piece * 512, 512)],
                                mats[1][:], src, start=False,
                                stop=(c0 == 1024))
                if p >= 1:
                    q = p - 1
                    z = zs[q]
                    w = wp.tile([G, 2048], BF16)
                    wst[q] = w
                    nc.scalar.activation(w[:], z[:], AFT.Exp)
                    nc.tensor.matmul(z[0:6, 0:512], w3s_t[:], w[:, 0:512],
                                     start=True, stop=False)
                    nc.tensor.matmul(z[0:6, 0:512], w3n_t[:], w[:, 1024:1536],
                                     start=False, stop=True)
                    nc.tensor.matmul(z[32:38, 0:512], w3s_t[:], w[:, 512:1024],
                                     start=True, stop=False)
                    nc.tensor.matmul(z[32:38, 0:512], w3n_t[:],
                                     w[:, 1536:2048], start=False, stop=True)
                    oba = obufp.tile([6, 512], F32, tag="oba",
                                     name=f"oba{q}")
                    obb = obufp.tile([6, 512], F32, tag="obb",
                                     name=f"obb{q}")
                    nc.vector.tensor_copy(oba[:], z[0:6, 0:512])
                    nc.vector.tensor_copy(obb[:], z[32:38, 0:512])
                    nc.sync.dma_start(out[0:6, bass.ts(q, 512)], oba[:])
                    nc.sync.dma_start(out[6:12, bass.ts(q, 512)], obb[:])

    nc.compile()
    return nc


def _box_max_z(coeffs):
    """Per-gaussian max of z(x) = k0 x0^2 + k1 x0 x1 + k2 x1^2 + k3 x0 +
    k4 x1 + k5 over the pixel box [-2,2]^2 (k0,k2 < 0: concave)."""
    k0, k1, k2, k3, k4, k5 = [coeffs[:, i] for i in range(6)]

    def ev(x0, x1):
        return (k0 * x0 * x0 + k1 * x0 * x1 + k2 * x1 * x1
                + k3 * x0 + k4 * x1 + k5)

    best = np.full(coeffs.shape[0], -np.inf)
    det = 4.0 * k0 * k2 - k1 * k1
    x0u = (-2.0 * k2 * k3 + k1 * k4) / det
    x1u = (-2.0 * k0 * k4 + k1 * k3) / det
    inside = (np.abs(x0u) <= 2.0) & (np.abs(x1u) <= 2.0) & (det > 0)
    best = np.where(inside, ev(np.clip(x0u, -2, 2), np.clip(x1u, -2, 2)), best)
    for s in (-2.0, 2.0):
        x1e = np.clip(-(k1 * s + k4) / (2.0 * k2), -2.0, 2.0)
        best = np.maximum(best, ev(s, x1e))
        x0e = np.clip(-(k1 * s + k3) / (2.0 * k0), -2.0, 2.0)
        best = np.maximum(best, ev(x0e, s))
    return best


def _host_prep(mu, chol, alpha, rgb, rotation, translation, projection, bg):
    # sort by camera distance in fp32 (matches reference argsort exactly)
    d32 = (mu.astype(np.float32) - translation.astype(np.float32)[None, :])
    dist = np.sqrt(np.sum(d32 * d32, axis=-1, dtype=np.float32))
    order = np.argsort(dist, kind="stable")
    mu = mu.astype(np.float64)[order]
    chol = chol.astype(np.float64)[order]
    alpha = alpha.astype(np.float64)[order]
    rgb = rgb.astype(np.float64)[order]
    rotation = rotation.astype(np.float64)
    translation = translation.astype(np.float64)
    projection = projection.astype(np.float64)
    bg = bg.astype(np.float64)

    inv_rot = rotation.T
    inv_trans = -inv_rot @ translation
    Lg = np.tril(chol) + 0.3 * np.eye(3)
    Sigma = np.einsum("gij,gkj->gik", Lg, Lg)
    mu_cam = np.einsum("ij,gj->gi", inv_rot, mu) + inv_trans
    mu2d = np.einsum("ij,gj->gi", projection, mu_cam)
    P_cam = projection @ inv_rot
    S2 = np.einsum("ij,gjk,lk->gil", P_cam, Sigma, P_cam) + 1e-4 * np.eye(2)
    det = S2[:, 0, 0] * S2[:, 1, 1] - S2[:, 0, 1] * S2[:, 1, 0]
    inv = np.empty((G, 2, 2))
    inv[:, 0, 0] = S2[:, 1, 1]
    inv[:, 0, 1] = -S2[:, 0, 1]
    inv[:, 1, 0] = -S2[:, 1, 0]
    inv[:, 1, 1] = S2[:, 0, 0]
    inv /= det[:, None, None]

    sp_ = np.logaddexp(0.0, alpha)
    wg = sp_ / (1.0 + sp_)
    color = rgb / (1.0 + np.abs(rgb))

    A = inv[:, 0, 0]
    Bc = inv[:, 0, 1] + inv[:, 1, 0]
    C = inv[:, 1, 1]
    m0, m1 = mu2d[:, 0], mu2d[:, 1]
    D = -2 * A * m0 - Bc * m1
    E = -Bc * m0 - 2 * C * m1
    F = A * m0 ** 2 + Bc * m0 * m1 + C * m1 ** 2
    coeffs = -0.5 * np.stack([A, Bc, C, D, E, F], axis=1)  # [G, 6]
    coeffs[:, 5] += np.log(wg)

    # strong = top 64 by max opacity over the pixel box; the rest get the
    # quadratic ln(1-a) approximation
    maxz = _box_max_z(coeffs)
    rk = np.argsort(-maxz, kind="stable")
    strong = np.sort(rk[:64])
    ns = np.sort(rk[64:])

    coefT = np.ascontiguousarray(coeffs.T).astype(np.float32)        # [6, G]
    ch = coefT.astype(BF)
    cl = (coefT - ch.astype(np.float32)).astype(BF)
    c18 = np.concatenate([ch, ch, cl], axis=0).astype(BF)            # [18, G]

    def mk_w1(sel):
        w = np.zeros((64, G), BF)
        w[0:18, 0:64] = c18[:, sel]
        w[32:50, 64:128] = c18[:, sel]
        return w

    def mk_tri(rows, cols, scale):
        m = (rows[:, None] > cols[None, :]).astype(np.float32) * scale
        t = np.zeros((G, G), np.float32)
        t[0:64, 0:64] = m
        t[64:128, 64:128] = m
        return t.astype(BF)

    def mk_w3(sel, colmb):
        w = np.zeros((G, 6), BF)
        w[0:64, 0:3] = colmb[sel]
        w[64:128, 3:6] = colmb[sel]
        return w

    colmb = (color - bg[None, :]).astype(BF)
    consts = {
        "w1s": mk_w1(strong),
        "w1n": mk_w1(ns),
        "tss": mk_tri(strong, strong, 1.0),
        "tsn": mk_tri(strong, ns, 1.0),
        "tns": mk_tri(ns, strong, -0.5),
        "tnn": mk_tri(ns, ns, -0.5),
        "w3s": mk_w3(strong, colmb),
        "w3n": mk_w3(ns, colmb),
    }
    return consts, bg.astype(np.float32)


def kernel(x, mu, chol, alpha, rgb, rotation, translation, projection,
           background_color):
    global _cached, LAST_EXEC_NS, LAST_RESULTS
    x = np.asarray(x, np.float32)
    consts, bg = _host_prep(
        np.asarray(mu), np.asarray(chol), np.asarray(alpha), np.asarray(rgb),
        np.asarray(rotation), np.asarray(translation), np.asarray(projection),
        np.asarray(background_color))

    xf = x.reshape(BN, 2).astype(np.float64)
    feat = np.empty((6, BN), np.float32)
    feat[0] = xf[:, 0] ** 2
    feat[1] = xf[:, 0] * xf[:, 1]
    feat[2] = xf[:, 1] ** 2
    feat[3] = xf[:, 0]
    feat[4] = xf[:, 1]
    feat[5] = 1.0
    fh = feat.astype(BF)
    fl = (feat - fh.astype(np.float32)).astype(BF)
    f18 = np.concatenate([fh, fl, fh], axis=0)                       # [18, BN]

    if _cached is None:
        _cached = _build()
    nc = _cached

    in_maps = []
    for k in range(NCORES):
        fc = f18[:, k * PPC:(k + 1) * PPC].reshape(18, PPC // 512, 512)
        f18p = np.zeros((64, PPC // 2), BF)
        f18p[0:18] = fc[:, 0::2].reshape(18, PPC // 2)
        f18p[32:50] = fc[:, 1::2].reshape(18, PPC // 2)
        m = {"f18p": f18p}
        m.update(consts)
        in_maps.append(m)

    kwargs = {}
    if PROFILE:
        kwargs = dict(trace=True)
    res = run_bass_kernel_spmd(nc, in_maps, core_ids=list(range(NCORES)),
                               **kwargs)
    LAST_EXEC_NS = res.exec_time_ns
    LAST_RESULTS = res
    # out [12, PPC/4]: rows (block q 0..3) x (color c 0..2); cols
    # (superstep p, 512): pixel = ((p*4 + q)*512 + i) within the core
    full = np.empty((BN, 3), np.float32)
    for k in range(NCORES):
        o = res.results[k]["out"].reshape(4, 3, NSUP, 512)
        full[k * PPC:(k + 1) * PPC] = (
            np.transpose(o, (2, 0, 3, 1)).reshape(PPC, 3))
    return (full + bg[None, :]).reshape(B, N, 3).astype(np.float32)


# revision 8
# speedup vs baseline: 1.4155x; 1.4155x over previous
"""Gaussian-splat blend kernel for 8 TRN2 NeuronCores — v2 (packed ln).

Math (per pixel p, gaussians sorted nearest-first):
  q_g(p)   = (x_p - mu2d_g)^T inv_g (x_p - mu2d_g)      quadratic in x
  a_g(p)   = w_g * exp(-q/2),  z = -q/2 + ln w
  out_c(p) = sum_g a_g * prod_{j>g}(1-a_j) * color_gc + prod_all(1-a_j)*bg_c

ScalarE (ACT) is the bottleneck: it runs 128 lanes x 1 col/cycle with no
dtype speedup, so cost = total COLUMNS.  v1 did 3 full transcendental
passes (exp, ln, exp) = 98.7us ACT busy.  v2 splits the gaussians by
per-gaussian max opacity over the pixel box:
  strong (top 64 by max_a): exact l = ln(1-a), PACKED 2-up -> the ln
    pass covers two 512-px blocks per 128-partition tile (half width).
  nonstrong (bottom 64, max_a <= ~0.37): l ~= -(a + a^2/2); the
    quadratic-truncation error gives rel-err ~8e-4 (gate 2e-2).  The
    polynomial is ONE idle-DVE op: p' = (a+2)*a = 2a+a^2, with the 1/2
    folded into -0.5 entries of the triangular matmul weights.
ACT passes: exp1 (1.0) + ln (0.5) + exp2 (1.0) = 2.5 vs 3.0, and each
superstep is one [128,2048] PSUM tile so exp1/exp2 are single wide
instructions (fewer fixed ~240ns/instr overheads).

Layout per 2048-px superstep: one PSUM tile z [128, 2048] f32:
  cols    0:512  piece A: strong slots, blocks b0 (rows 0:64) | b1 (64:128)
  cols  512:1024 piece B: strong, b2 | b3
  cols 1024:1536 piece C: nonstrong, b0 | b1
  cols 1536:2048 piece D: nonstrong, b2 | b3
mm1: 4 block-diagonal matmuls (18-row error-compensated bf16 coeffs at
strip rows 0:18 / 32:50 of the v1 f18p feature layout).
ACT exp1: a = exp(z) [128,2048] f32.  ACT ln: l = ln(1-a[:,0:1024]) bf16.
DVE: p' = (a_ns+2)*a_ns bf16.  mm2: 8 matmuls accumulate S onto z
(blockdiag strict-order tri matrices; -0.5 weights for p').  ACT exp2:
w = exp(z+S) bf16.  mm3: 4 matmuls -> z[0:12,0:512] (freed rows), DVE
copy -> SBUF, DMA out.  exp2/mm3/copy are skewed one superstep so ACT
fills the mm2 latency with the next exp1.  Host adds bg and unpacks.
"""

import numpy as np
import ml_dtypes

import concourse.bass as bass
import concourse.bacc as bacc
import concourse.mybir as mybir
import concourse.tile as tile
from concourse.bass_utils import run_bass_kernel_spmd

G = 128
B = 4
N = 65536
BN = B * N
NCORES = 8
PPC = BN // NCORES          # pixels per core = 32768
SUPPX = 2048                # pixels per superstep
NSUP = PPC // SUPPX         # 16
FW = 1024                   # f18p cols per superstep (2 blocks packed/col)

F32 = mybir.dt.float32
BF16 = mybir.dt.bfloat16
AFT = mybir.ActivationFunctionType
ALU = mybir.AluOpType
BF = ml_dtypes.bfloat16

PROFILE = False
LAST_EXEC_NS = None
LAST_RESULTS = None

_cached = None


def _patch_act_tables():
    """Force every activation onto the one table set that has BOTH Exp and
    Ln ("natural_log_exp_and_others") — otherwise the table-load pass
    alternates sets and burns ~1.3us per ACT_TABLE_LOAD, once per tile."""
    if getattr(bacc, "_act_tables_patched", False):
        return
    orig = bacc.get_activation_tables

    def only_nle(arch):
        tabs = orig(arch)
        return {
            name: (fns if name == "natural_log_exp_and_others" else set())
            for name, fns in tabs.items()
        }

    bacc.get_activation_tables = only_nle
    bacc._act_tables_patched = True


def _build():
    _patch_act_tables()
    nc = bacc.Bacc("TRN2", target_bir_lowering=False, debug=False,
                   num_devices=NCORES)
    f18p = nc.dram_tensor("f18p", [64, PPC // 2], BF16, kind="ExternalInput")
    w1s = nc.dram_tensor("w1s", [64, G], BF16, kind="ExternalInput")
    w1n = nc.dram_tensor("w1n", [64, G], BF16, kind="ExternalInput")
    tss = nc.dram_tensor("tss", [G, G], BF16, kind="ExternalInput")
    tsn = nc.dram_tensor("tsn", [G, G], BF16, kind="ExternalInput")
    tns = nc.dram_tensor("tns", [G, G], BF16, kind="ExternalInput")
    tnn = nc.dram_tensor("tnn", [G, G], BF16, kind="ExternalInput")
    w3s = nc.dram_tensor("w3s", [G, 6], BF16, kind="ExternalInput")
    w3n = nc.dram_tensor("w3n", [G, 6], BF16, kind="ExternalInput")
    out = nc.dram_tensor("out", [12, PPC // 4], F32, kind="ExternalOutput")

    with tile.TileContext(nc) as tc:
        with (
            tc.tile_pool(name="const", bufs=1) as constp,
            tc.tile_pool(name="featp", bufs=3) as featp,
            tc.tile_pool(name="zsp", bufs=2, space="PSUM") as zsp,
            tc.tile_pool(name="znp", bufs=2, space="PSUM") as znp,
            tc.tile_pool(name="aa", bufs=3) as ap_,
            tc.tile_pool(name="lp", bufs=3) as lp,
            tc.tile_pool(name="pp", bufs=3) as pp,
            tc.tile_pool(name="wp", bufs=3) as wp,
            tc.tile_pool(name="obuf", bufs=4) as obufp,
        ):
            # dependency-free dummy activation: pulls the ~1.3us
            # ACT_TABLE_LOAD into the idle DMA-wait head
            dummy = constp.tile([1, 8], F32)
            nc.gpsimd.memset(dummy[:], 0.0)
            nc.scalar.activation(dummy[:], dummy[:], AFT.Exp)

            fbufs = [featp.tile([64, FW], BF16, tag="fbuf", name=f"fbuf{i}")
                     for i in range(NSUP)]
            # first feature block + mm1 weights on the sync queue, the rest
            # on gpsimd so descriptor-gens run in parallel
            nc.sync.dma_start(fbufs[0][:], f18p[:, bass.ts(0, FW)])
            w1s_t = constp.tile([64, G], BF16)
            nc.sync.dma_start(w1s_t[:], w1s[:])
            w1n_t = constp.tile([64, G], BF16)
            nc.sync.dma_start(w1n_t[:], w1n[:])
            tss_t = constp.tile([G, G], BF16)
            nc.gpsimd.dma_start(tss_t[:], tss[:])
            tsn_t = constp.tile([G, G], BF16)
            nc.gpsimd.dma_start(tsn_t[:], tsn[:])
            tns_t = constp.tile([G, G], BF16)
            nc.gpsimd.dma_start(tns_t[:], tns[:])
            tnn_t = constp.tile([G, G], BF16)
            nc.gpsimd.dma_start(tnn_t[:], tnn[:])
            w3s_t = constp.tile([G, 6], BF16)
            nc.gpsimd.dma_start(w3s_t[:], w3s[:])
            w3n_t = constp.tile([G, 6], BF16)
            nc.gpsimd.dma_start(w3n_t[:], w3n[:])

            zss = [None] * NSUP
            zns = [None] * NSUP
            for p in range(NSUP + 1):
                if p < NSUP:
                    if p + 1 < NSUP:
                        nc.gpsimd.dma_start(fbufs[p + 1][:],
                                            f18p[:, bass.ts(p + 1, FW)])
                    fb = fbufs[p]
                    zs = zsp.tile([G, 1024], F32)
                    zn = znp.tile([G, 1024], F32)
                    zss[p] = zs
                    zns[p] = zn
                    nc.tensor.matmul(zs[:, 0:512], w1s_t[:], fb[:, 0:512],
                                     start=True, stop=False)
                    nc.tensor.matmul(zs[:, 512:1024], w1s_t[:],
                                     fb[:, 512:1024], start=True, stop=False)
                    nc.tensor.matmul(zn[:, 0:512], w1n_t[:], fb[:, 0:512],
                                     start=True, stop=False)
                    nc.tensor.matmul(zn[:, 512:1024], w1n_t[:],
                                     fb[:, 512:1024], start=True, stop=False)
                    a = ap_.tile([G, 2048], F32)
                    nc.scalar.activation(a[:, 0:1024], zs[:], AFT.Exp)
                    nc.scalar.activation(a[:, 1024:2048], zn[:], AFT.Exp)
                    l = lp.tile([G, 1024], BF16)
                    nc.scalar.activation(l[:], a[:, 0:1024], AFT.Ln,
                                         bias=1.0, scale=-1.0)
                    pq = pp.tile([G, 1024], BF16)
                    nc.vector.scalar_tensor_tensor(
                        pq[:], a[:, 1024:2048], 2.0, a[:, 1024:2048],
                        ALU.add, ALU.mult)
                    for piece in range(2):
                        sl = l[:, bass.ts(piece, 512)]
                        sp = pq[:, bass.ts(piece, 512)]
                        dst = bass.ds(piece * 512, 512)
                        nc.tensor.matmul(zs[:, dst], tss_t[:], sl,
                                         start=False, stop=False)
                        nc.tensor.matmul(zs[:, dst], tns_t[:], sp,
                                         start=False, stop=True)
                        nc.tensor.matmul(zn[:, dst], tsn_t[:], sl,
                                         start=False, stop=False)
                        nc.tensor.matmul(zn[:, dst], tnn_t[:], sp,
                                         start=False, stop=True)
                if p >= 1:
                    q = p - 1
                    zs, zn = zss[q], zns[q]
                    w = wp.tile([G, 2048], BF16)
                    nc.scalar.activation(w[:, 0:1024], zs[:], AFT.Exp)
                    nc.scalar.activation(w[:, 1024:2048], zn[:], AFT.Exp)
                    nc.tensor.matmul(zn[0:6, 0:512], w3s_t[:], w[:, 0:512],
                                     start=True, stop=False)
                    nc.tensor.matmul(zn[0:6, 0:512], w3n_t[:], w[:, 1024:1536],
                                     start=False, stop=True)
                    nc.tensor.matmul(zn[32:38, 0:512], w3s_t[:],
                                     w[:, 512:1024], start=True, stop=False)
                    nc.tensor.matmul(zn[32:38, 0:512], w3n_t[:],
                                     w[:, 1536:2048], start=False, stop=True)
                    oba = obufp.tile([6, 512], F32, tag="oba",
                                     name=f"oba{q}")
                    obb = obufp.tile([6, 512], F32, tag="obb",
                                     name=f"obb{q}")
                    nc.vector.tensor_copy(oba[:], zn[0:6, 0:512])
                    nc.vector.tensor_copy(obb[:], zn[32:38, 0:512])
                    nc.sync.dma_start(out[0:6, bass.ts(q, 512)], oba[:])
                    nc.sync.dma_start(out[6:12, bass.ts(q, 512)], obb[:])

    nc.compile()
    return nc


def _box_max_z(coeffs):
    """Per-gaussian max of z(x) = k0 x0^2 + k1 x0 x1 + k2 x1^2 + k3 x0 +
    k4 x1 + k5 over the pixel box [-2,2]^2 (k0,k2 < 0: concave)."""
    k0, k1, k2, k3, k4, k5 = [coeffs[:, i] for i in range(6)]

    def ev(x0, x1):
        return (k0 * x0 * x0 + k1 * x0 * x1 + k2 * x1 * x1
                + k3 * x0 + k4 * x1 + k5)

    best = np.full(coeffs.shape[0], -np.inf)
    det = 4.0 * k0 * k2 - k1 * k1
    x0u = (-2.0 * k2 * k3 + k1 * k4) / det
    x1u = (-2.0 * k0 * k4 + k1 * k3) / det
    inside = (np.abs(x0u) <= 2.0) & (np.abs(x1u) <= 2.0) & (det > 0)
    best = np.where(inside, ev(np.clip(x0u, -2, 2), np.clip(x1u, -2, 2)), best)
    for s in (-2.0, 2.0):
        x1e = np.clip(-(k1 * s + k4) / (2.0 * k2), -2.0, 2.0)
        best = np.maximum(best, ev(s, x1e))
        x0e = np.clip(-(k1 * s + k3) / (2.0 * k0), -2.0, 2.0)
        best = np.maximum(best, ev(x0e, s))
    return best


def _host_prep(mu, chol, alpha, rgb, rotation, translation, projection, bg):
    # sort by camera distance in fp32 (matches reference argsort exactly)
    d32 = (mu.astype(np.float32) - translation.astype(np.float32)[None, :])
    dist = np.sqrt(np.sum(d32 * d32, axis=-1, dtype=np.float32))
    order = np.argsort(dist, kind="stable")
    mu = mu.astype(np.float64)[order]
    chol = chol.astype(np.float64)[order]
    alpha = alpha.astype(np.float64)[order]
    rgb = rgb.astype(np.float64)[order]
    rotation = rotation.astype(np.float64)
    translation = translation.astype(np.float64)
    projection = projection.astype(np.float64)
    bg = bg.astype(np.float64)

    inv_rot = rotation.T
    inv_trans = -inv_rot @ translation
    Lg = np.tril(chol) + 0.3 * np.eye(3)
    Sigma = np.einsum("gij,gkj->gik", Lg, Lg)
    mu_cam = np.einsum("ij,gj->gi", inv_rot, mu) + inv_trans
    mu2d = np.einsum("ij,gj->gi", projection, mu_cam)
    P_cam = projection @ inv_rot
    S2 = np.einsum("ij,gjk,lk->gil", P_cam, Sigma, P_cam) + 1e-4 * np.eye(2)
    det = S2[:, 0, 0] * S2[:, 1, 1] - S2[:, 0, 1] * S2[:, 1, 0]
    inv = np.empty((G, 2, 2))
    inv[:, 0, 0] = S2[:, 1, 1]
    inv[:, 0, 1] = -S2[:, 0, 1]
    inv[:, 1, 0] = -S2[:, 1, 0]
    inv[:, 1, 1] = S2[:, 0, 0]
    inv /= det[:, None, None]

    sp_ = np.logaddexp(0.0, alpha)
    wg = sp_ / (1.0 + sp_)
    color = rgb / (1.0 + np.abs(rgb))

    A = inv[:, 0, 0]
    Bc = inv[:, 0, 1] + inv[:, 1, 0]
    C = inv[:, 1, 1]
    m0, m1 = mu2d[:, 0], mu2d[:, 1]
    D = -2 * A * m0 - Bc * m1
    E = -Bc * m0 - 2 * C * m1
    F = A * m0 ** 2 + Bc * m0 * m1 + C * m1 ** 2
    coeffs = -0.5 * np.stack([A, Bc, C, D, E, F], axis=1)  # [G, 6]
    coeffs[:, 5] += np.log(wg)

    # strong = top 64 by max opacity over the pixel box; the rest get the
    # quadratic ln(1-a) approximation
    maxz = _box_max_z(coeffs)
    rk = np.argsort(-maxz, kind="stable")
    strong = np.sort(rk[:64])
    ns = np.sort(rk[64:])

    coefT = np.ascontiguousarray(coeffs.T).astype(np.float32)        # [6, G]
    ch = coefT.astype(BF)
    cl = (coefT - ch.astype(np.float32)).astype(BF)
    c18 = np.concatenate([ch, ch, cl], axis=0).astype(BF)            # [18, G]

    def mk_w1(sel):
        w = np.zeros((64, G), BF)
        w[0:18, 0:64] = c18[:, sel]
        w[32:50, 64:128] = c18[:, sel]
        return w

    def mk_tri(rows, cols, scale):
        m = (rows[:, None] > cols[None, :]).astype(np.float32) * scale
        t = np.zeros((G, G), np.float32)
        t[0:64, 0:64] = m
        t[64:128, 64:128] = m
        return t.astype(BF)

    def mk_w3(sel, colmb):
        w = np.zeros((G, 6), BF)
        w[0:64, 0:3] = colmb[sel]
        w[64:128, 3:6] = colmb[sel]
        return w

    colmb = (color - bg[None, :]).astype(BF)
    consts = {
        "w1s": mk_w1(strong),
        "w1n": mk_w1(ns),
        "tss": mk_tri(strong, strong, 1.0),
        "tsn": mk_tri(strong, ns, 1.0),
        "tns": mk_tri(ns, strong, -0.5),
        "tnn": mk_tri(ns, ns, -0.5),
        "w3s": mk_w3(strong, colmb),
        "w3n": mk_w3(ns, colmb),
    }
    return consts, bg.astype(np.float32)


def kernel(x, mu, chol, alpha, rgb, rotation, translation, projection,
           background_color):
    global _cached, LAST_EXEC_NS, LAST_RESULTS
    x = np.asarray(x, np.float32)
    consts, bg = _host_prep(
        np.asarray(mu), np.asarray(chol), np.asarray(alpha), np.asarray(rgb),
        np.asarray(rotation), np.asarray(translation), np.asarray(projection),
        np.asarray(background_color))

    xf = x.reshape(BN, 2).astype(np.float64)
    feat = np.empty((6, BN), np.float32)
    feat[0] = xf[:, 0] ** 2
    feat[1] = xf[:, 0] * xf[:, 1]
    feat[2] = xf[:, 1] ** 2
    feat[3] = xf[:, 0]
    feat[4] = xf[:, 1]
    feat[5] = 1.0
    fh = feat.astype(BF)
    fl = (feat - fh.astype(np.float32)).astype(BF)
    f18 = np.concatenate([fh, fl, fh], axis=0)                       # [18, BN]

    if _cached is None:
        _cached = _build()
    nc = _cached

    in_maps = []
    for k in range(NCORES):
        fc = f18[:, k * PPC:(k + 1) * PPC].reshape(18, PPC // 512, 512)
        f18p = np.zeros((64, PPC // 2), BF)
        f18p[0:18] = fc[:, 0::2].reshape(18, PPC // 2)
        f18p[32:50] = fc[:, 1::2].reshape(18, PPC // 2)
        m = {"f18p": f18p}
        m.update(consts)
        in_maps.append(m)

    kwargs = {}
    if PROFILE:
        kwargs = dict(trace=True)
    res = run_bass_kernel_spmd(nc, in_maps, core_ids=list(range(NCORES)),
                               **kwargs)
    LAST_EXEC_NS = res.exec_time_ns
    LAST_RESULTS = res
    # out [12, PPC/4]: rows (block q 0..3) x (color c 0..2); cols
    # (superstep p, 512): pixel = ((p*4 + q)*512 + i) within the core
    full = np.empty((BN, 3), np.float32)
    for k in range(NCORES):
        o = res.results[k]["out"].reshape(4, 3, NSUP, 512)
        full[k * PPC:(k + 1) * PPC] = (
            np.transpose(o, (2, 0, 3, 1)).reshape(PPC, 3))
    return (full + bg[None, :]).reshape(B, N, 3).astype(np.float32)


# revision 10
# speedup vs baseline: 1.4445x; 1.0205x over previous
"""Gaussian-splat blend kernel for 8 TRN2 NeuronCores — v2 (packed ln).

Math (per pixel p, gaussians sorted nearest-first):
  q_g(p)   = (x_p - mu2d_g)^T inv_g (x_p - mu2d_g)      quadratic in x
  a_g(p)   = w_g * exp(-q/2),  z = -q/2 + ln w
  out_c(p) = sum_g a_g * prod_{j>g}(1-a_j) * color_gc + prod_all(1-a_j)*bg_c

ScalarE (ACT) is the bottleneck: it runs 128 lanes x 1 col/cycle with no
dtype speedup, so cost = total COLUMNS.  v1 did 3 full transcendental
passes (exp, ln, exp) = 98.7us ACT busy.  v2 splits the gaussians by
per-gaussian max opacity over the pixel box:
  strong (top 64 by max_a): exact l = ln(1-a), PACKED 2-up -> the ln
    pass covers two 512-px blocks per 128-partition tile (half width).
  nonstrong (bottom 64, max_a <= ~0.37): l ~= -(a + a^2/2); the
    quadratic-truncation error gives rel-err ~8e-4 (gate 2e-2).  The
    polynomial is ONE idle-DVE op: p' = (a+2)*a = 2a+a^2, with the 1/2
    folded into -0.5 entries of the triangular matmul weights.
ACT passes: exp1 (1.0) + ln (0.5) + exp2 (1.0) = 2.5 vs 3.0, and each
superstep is one [128,2048] PSUM tile so exp1/exp2 are single wide
instructions (fewer fixed ~240ns/instr overheads).

Layout per 2048-px superstep: one PSUM tile z [128, 2048] f32:
  cols    0:512  piece A: strong slots, blocks b0 (rows 0:64) | b1 (64:128)
  cols  512:1024 piece B: strong, b2 | b3
  cols 1024:1536 piece C: nonstrong, b0 | b1
  cols 1536:2048 piece D: nonstrong, b2 | b3
mm1: 4 block-diagonal matmuls (18-row error-compensated bf16 coeffs at
strip rows 0:18 / 32:50 of the v1 f18p feature layout).
ACT exp1: a = exp(z) [128,2048] f32.  ACT ln: l = ln(1-a[:,0:1024]) bf16.
DVE: p' = (a_ns+2)*a_ns bf16.  mm2: 8 matmuls accumulate S onto z
(blockdiag strict-order tri matrices; -0.5 weights for p').  ACT exp2:
w = exp(z+S) bf16.  mm3: 4 matmuls -> z[0:12,0:512] (freed rows), DVE
copy -> SBUF, DMA out.  exp2/mm3/copy are skewed one superstep so ACT
fills the mm2 latency with the next exp1.  Host adds bg and unpacks.
"""

import numpy as np
import ml_dtypes

import concourse.bass as bass
import concourse.bacc as bacc
import concourse.mybir as mybir
import concourse.tile as tile
from concourse.bass_utils import run_bass_kernel_spmd

G = 128
B = 4
N = 65536
BN = B * N
NCORES = 8
PPC = BN // NCORES          # pixels per core = 32768
SUPPX = 2048                # pixels per superstep
NSUP = PPC // SUPPX         # 16
FW = 1024                   # f18p cols per superstep (2 blocks packed/col)

F32 = mybir.dt.float32
BF16 = mybir.dt.bfloat16
AFT = mybir.ActivationFunctionType
ALU = mybir.AluOpType
BF = ml_dtypes.bfloat16

PROFILE = False
LAST_EXEC_NS = None
LAST_RESULTS = None

_cached = None


def _patch_act_tables():
    """Force every activation onto the one table set that has BOTH Exp and
    Ln ("natural_log_exp_and_others") — otherwise the table-load pass
    alternates sets and burns ~1.3us per ACT_TABLE_LOAD, once per tile."""
    if getattr(bacc, "_act_tables_patched", False):
        return
    orig = bacc.get_activation_tables

    def only_nle(arch):
        tabs = orig(arch)
        return {
            name: (fns if name == "natural_log_exp_and_others" else set())
            for name, fns in tabs.items()
        }

    bacc.get_activation_tables = only_nle
    bacc._act_tables_patched = True


def _build():
    _patch_act_tables()
    nc = bacc.Bacc("TRN2", target_bir_lowering=False, debug=False,
                   num_devices=NCORES)
    f18p = nc.dram_tensor("f18p", [64, PPC // 2], BF16, kind="ExternalInput")
    w1s = nc.dram_tensor("w1s", [64, G], BF16, kind="ExternalInput")
    w1n = nc.dram_tensor("w1n", [64, G], BF16, kind="ExternalInput")
    tss = nc.dram_tensor("tss", [G, G], BF16, kind="ExternalInput")
    tsn = nc.dram_tensor("tsn", [G, G], BF16, kind="ExternalInput")
    tns = nc.dram_tensor("tns", [G, G], BF16, kind="ExternalInput")
    tnn = nc.dram_tensor("tnn", [G, G], BF16, kind="ExternalInput")
    w3s = nc.dram_tensor("w3s", [G, 6], BF16, kind="ExternalInput")
    w3n = nc.dram_tensor("w3n", [G, 6], BF16, kind="ExternalInput")
    out = nc.dram_tensor("out", [6, PPC // 2], F32, kind="ExternalOutput")

    with tile.TileContext(nc) as tc:
        with (
            tc.tile_pool(name="const", bufs=1) as constp,
            tc.tile_pool(name="featp", bufs=3) as featp,
            tc.tile_pool(name="zsp", bufs=2, space="PSUM") as zsp,
            tc.tile_pool(name="znp", bufs=2, space="PSUM") as znp,
            tc.tile_pool(name="aa", bufs=3) as ap_,
            tc.tile_pool(name="lp", bufs=3) as lp,
            tc.tile_pool(name="pp", bufs=3) as pp,
            tc.tile_pool(name="wp", bufs=3) as wp,
            tc.tile_pool(name="obuf", bufs=4) as obufp,
        ):
            # dependency-free dummy activation: pulls the ~1.3us
            # ACT_TABLE_LOAD into the idle DMA-wait head
            dummy = constp.tile([1, 8], F32)
            nc.gpsimd.memset(dummy[:], 0.0)
            nc.scalar.activation(dummy[:], dummy[:], AFT.Exp)

            fbufs = [featp.tile([64, FW], BF16, tag="fbuf", name=f"fbuf{i}")
                     for i in range(NSUP)]
            # first feature block + mm1 weights on the sync queue, the rest
            # on gpsimd so descriptor-gens run in parallel
            nc.sync.dma_start(fbufs[0][:], f18p[:, bass.ts(0, FW)])
            w1s_t = constp.tile([64, G], BF16)
            nc.sync.dma_start(w1s_t[:], w1s[:])
            w1n_t = constp.tile([64, G], BF16)
            nc.sync.dma_start(w1n_t[:], w1n[:])
            tss_t = constp.tile([G, G], BF16)
            nc.gpsimd.dma_start(tss_t[:], tss[:])
            tsn_t = constp.tile([G, G], BF16)
            nc.gpsimd.dma_start(tsn_t[:], tsn[:])
            tns_t = constp.tile([G, G], BF16)
            nc.gpsimd.dma_start(tns_t[:], tns[:])
            tnn_t = constp.tile([G, G], BF16)
            nc.gpsimd.dma_start(tnn_t[:], tnn[:])
            w3s_t = constp.tile([G, 6], BF16)
            nc.gpsimd.dma_start(w3s_t[:], w3s[:])
            w3n_t = constp.tile([G, 6], BF16)
            nc.gpsimd.dma_start(w3n_t[:], w3n[:])

            zss = [None] * NSUP
            zns = [None] * NSUP
            for p in range(NSUP + 1):
                if p < NSUP:
                    if p + 1 < NSUP:
                        nc.gpsimd.dma_start(fbufs[p + 1][:],
                                            f18p[:, bass.ts(p + 1, FW)])
                    fb = fbufs[p]
                    zs = zsp.tile([G, 1024], F32)
                    zn = znp.tile([G, 1024], F32)
                    zss[p] = zs
                    zns[p] = zn
                    for h in range(2):
                        fph = fb[:, bass.ts(h, 512)]
                        dst = bass.ds(h * 512, 512)
                        nc.tensor.matmul(zs[:, dst], w1s_t[:], fph,
                                         start=True, stop=False)
                        nc.tensor.matmul(zn[:, dst], w1n_t[:], fph,
                                         start=True, stop=False)
                    a = ap_.tile([G, 2048], F32)
                    nc.scalar.activation(a[:, 0:1024], zs[:], AFT.Exp)
                    nc.scalar.activation(a[:, 1024:2048], zn[:], AFT.Exp)
                    l = lp.tile([G, 1024], BF16)
                    nc.scalar.activation(l[:], a[:, 0:1024], AFT.Ln,
                                         bias=1.0, scale=-1.0)
                    pq = pp.tile([G, 1024], BF16)
                    nc.vector.scalar_tensor_tensor(
                        pq[:], a[:, 1024:2048], 2.0, a[:, 1024:2048],
                        ALU.add, ALU.mult)
                    for h in range(2):
                        sl = l[:, bass.ts(h, 512)]
                        sp = pq[:, bass.ts(h, 512)]
                        dst = bass.ds(h * 512, 512)
                        nc.tensor.matmul(zs[:, dst], tss_t[:], sl,
                                         start=False, stop=False)
                        nc.tensor.matmul(zs[:, dst], tns_t[:], sp,
                                         start=False, stop=True)
                        nc.tensor.matmul(zn[:, dst], tsn_t[:], sl,
                                         start=False, stop=False)
                        nc.tensor.matmul(zn[:, dst], tnn_t[:], sp,
                                         start=False, stop=True)
                if p >= 1:
                    q = p - 1
                    zs, zn = zss[q], zns[q]
                    w = wp.tile([G, 2048], BF16)
                    nc.scalar.activation(w[:, 0:1024], zs[:], AFT.Exp)
                    nc.scalar.activation(w[:, 1024:2048], zn[:], AFT.Exp)
                    for h in range(2):
                        dst = bass.ds(h * 512, 512)
                        nc.tensor.matmul(zn[0:6, dst], w3s_t[:],
                                         w[:, bass.ds(h * 512, 512)],
                                         start=True, stop=False)
                        nc.tensor.matmul(zn[0:6, dst], w3n_t[:],
                                         w[:, bass.ds(1024 + h * 512, 512)],
                                         start=False, stop=True)
                    ob = obufp.tile([6, 1024], F32, tag="ob", name=f"ob{q}")
                    nc.vector.tensor_copy(ob[:], zn[0:6, 0:1024])
                    nc.sync.dma_start(out[:, bass.ts(q, 1024)], ob[:])

    nc.compile()
    return nc


def _box_max_z(coeffs):
    """Per-gaussian max of z(x) = k0 x0^2 + k1 x0 x1 + k2 x1^2 + k3 x0 +
    k4 x1 + k5 over the pixel box [-2,2]^2 (k0,k2 < 0: concave)."""
    k0, k1, k2, k3, k4, k5 = [coeffs[:, i] for i in range(6)]

    def ev(x0, x1):
        return (k0 * x0 * x0 + k1 * x0 * x1 + k2 * x1 * x1
                + k3 * x0 + k4 * x1 + k5)

    best = np.full(coeffs.shape[0], -np.inf)
    det = 4.0 * k0 * k2 - k1 * k1
    x0u = (-2.0 * k2 * k3 + k1 * k4) / det
    x1u = (-2.0 * k0 * k4 + k1 * k3) / det
    inside = (np.abs(x0u) <= 2.0) & (np.abs(x1u) <= 2.0) & (det > 0)
    best = np.where(inside, ev(np.clip(x0u, -2, 2), np.clip(x1u, -2, 2)), best)
    for s in (-2.0, 2.0):
        x1e = np.clip(-(k1 * s + k4) / (2.0 * k2), -2.0, 2.0)
        best = np.maximum(best, ev(s, x1e))
        x0e = np.clip(-(k1 * s + k3) / (2.0 * k0), -2.0, 2.0)
        best = np.maximum(best, ev(x0e, s))
    return best


def _host_prep(mu, chol, alpha, rgb, rotation, translation, projection, bg):
    # sort by camera distance in fp32 (matches reference argsort exactly)
    d32 = (mu.astype(np.float32) - translation.astype(np.float32)[None, :])
    dist = np.sqrt(np.sum(d32 * d32, axis=-1, dtype=np.float32))
    order = np.argsort(dist, kind="stable")
    mu = mu.astype(np.float64)[order]
    chol = chol.astype(np.float64)[order]
    alpha = alpha.astype(np.float64)[order]
    rgb = rgb.astype(np.float64)[order]
    rotation = rotation.astype(np.float64)
    translation = translation.astype(np.float64)
    projection = projection.astype(np.float64)
    bg = bg.astype(np.float64)

    inv_rot = rotation.T
    inv_trans = -inv_rot @ translation
    Lg = np.tril(chol) + 0.3 * np.eye(3)
    Sigma = np.einsum("gij,gkj->gik", Lg, Lg)
    mu_cam = np.einsum("ij,gj->gi", inv_rot, mu) + inv_trans
    mu2d = np.einsum("ij,gj->gi", projection, mu_cam)
    P_cam = projection @ inv_rot
    S2 = np.einsum("ij,gjk,lk->gil", P_cam, Sigma, P_cam) + 1e-4 * np.eye(2)
    det = S2[:, 0, 0] * S2[:, 1, 1] - S2[:, 0, 1] * S2[:, 1, 0]
    inv = np.empty((G, 2, 2))
    inv[:, 0, 0] = S2[:, 1, 1]
    inv[:, 0, 1] = -S2[:, 0, 1]
    inv[:, 1, 0] = -S2[:, 1, 0]
    inv[:, 1, 1] = S2[:, 0, 0]
    inv /= det[:, None, None]

    sp_ = np.logaddexp(0.0, alpha)
    wg = sp_ / (1.0 + sp_)
    color = rgb / (1.0 + np.abs(rgb))

    A = inv[:, 0, 0]
    Bc = inv[:, 0, 1] + inv[:, 1, 0]
    C = inv[:, 1, 1]
    m0, m1 = mu2d[:, 0], mu2d[:, 1]
    D = -2 * A * m0 - Bc * m1
    E = -Bc * m0 - 2 * C * m1
    F = A * m0 ** 2 + Bc * m0 * m1 + C * m1 ** 2
    coeffs = -0.5 * np.stack([A, Bc, C, D, E, F], axis=1)  # [G, 6]
    coeffs[:, 5] += np.log(wg)

    # strong = top 64 by max opacity over the pixel box; the rest get the
    # quadratic ln(1-a) approximation
    maxz = _box_max_z(coeffs)
    rk = np.argsort(-maxz, kind="stable")
    strong = np.sort(rk[:64])
    ns = np.sort(rk[64:])

    coefT = np.ascontiguousarray(coeffs.T).astype(np.float32)        # [6, G]
    ch = coefT.astype(BF)
    cl = (coefT - ch.astype(np.float32)).astype(BF)
    c18 = np.concatenate([ch, ch, cl], axis=0).astype(BF)            # [18, G]

    def mk_w1(sel):
        w = np.zeros((64, G), BF)
        w[0:18, 0:64] = c18[:, sel]
        w[32:50, 64:128] = c18[:, sel]
        return w

    def mk_tri(rows, cols, scale):
        m = (rows[:, None] > cols[None, :]).astype(np.float32) * scale
        t = np.zeros((G, G), np.float32)
        t[0:64, 0:64] = m
        t[64:128, 64:128] = m
        return t.astype(BF)

    def mk_w3(sel, colmb):
        w = np.zeros((G, 6), BF)
        w[0:64, 0:3] = colmb[sel]
        w[64:128, 3:6] = colmb[sel]
        return w

    colmb = (color - bg[None, :]).astype(BF)
    consts = {
        "w1s": mk_w1(strong),
        "w1n": mk_w1(ns),
        "tss": mk_tri(strong, strong, 1.0),
        "tsn": mk_tri(strong, ns, 1.0),
        "tns": mk_tri(ns, strong, -0.5),
        "tnn": mk_tri(ns, ns, -0.5),
        "w3s": mk_w3(strong, colmb),
        "w3n": mk_w3(ns, colmb),
    }
    return consts, bg.astype(np.float32)


def kernel(x, mu, chol, alpha, rgb, rotation, translation, projection,
           background_color):
    global _cached, LAST_EXEC_NS, LAST_RESULTS
    x = np.asarray(x, np.float32)
    consts, bg = _host_prep(
        np.asarray(mu), np.asarray(chol), np.asarray(alpha), np.asarray(rgb),
        np.asarray(rotation), np.asarray(translation), np.asarray(projection),
        np.asarray(background_color))

    xf = x.reshape(BN, 2).astype(np.float64)
    feat = np.empty((6, BN), np.float32)
    feat[0] = xf[:, 0] ** 2
    feat[1] = xf[:, 0] * xf[:, 1]
    feat[2] = xf[:, 1] ** 2
    feat[3] = xf[:, 0]
    feat[4] = xf[:, 1]
    feat[5] = 1.0
    fh = feat.astype(BF)
    fl = (feat - fh.astype(np.float32)).astype(BF)
    f18 = np.concatenate([fh, fl, fh], axis=0)                       # [18, BN]

    if _cached is None:
        _cached = _build()
    nc = _cached

    in_maps = []
    for k in range(NCORES):
        fc = f18[:, k * PPC:(k + 1) * PPC].reshape(18, PPC // 512, 512)
        f18p = np.zeros((64, PPC // 2), BF)
        f18p[0:18] = fc[:, 0::2].reshape(18, PPC // 2)
        f18p[32:50] = fc[:, 1::2].reshape(18, PPC // 2)
        m = {"f18p": f18p}
        m.update(consts)
        in_maps.append(m)

    kwargs = {}
    if PROFILE:
        kwargs = dict(trace=True)
    res = run_bass_kernel_spmd(nc, in_maps, core_ids=list(range(NCORES)),
                               **kwargs)
    LAST_EXEC_NS = res.exec_time_ns
    LAST_RESULTS = res
    # out [6, PPC/2]: row r = q2*3 + c (q2: even/odd 512-block); col =
    # p*1024 + h*512 + i  ->  pixel = 2048p + (2h + q2)*512 + i
    full = np.empty((BN, 3), np.float32)
    for k in range(NCORES):
        o = res.results[k]["out"].reshape(2, 3, NSUP, 2, 512)
        full[k * PPC:(k + 1) * PPC] = (
            np.transpose(o, (2, 3, 0, 4, 1)).reshape(PPC, 3))
    return (full + bg[None, :]).reshape(B, N, 3).astype(np.float32)


# revision 12
# speedup vs baseline: 1.4989x; 1.0376x over previous
"""Gaussian-splat blend kernel for 8 TRN2 NeuronCores — v2 (packed ln).

Math (per pixel p, gaussians sorted nearest-first):
  q_g(p)   = (x_p - mu2d_g)^T inv_g (x_p - mu2d_g)      quadratic in x
  a_g(p)   = w_g * exp(-q/2),  z = -q/2 + ln w
  out_c(p) = sum_g a_g * prod_{j>g}(1-a_j) * color_gc + prod_all(1-a_j)*bg_c

ScalarE (ACT) is the bottleneck: it runs 128 lanes x 1 col/cycle with no
dtype speedup, so cost = total COLUMNS.  v1 did 3 full transcendental
passes (exp, ln, exp) = 98.7us ACT busy.  v2 splits the gaussians by
per-gaussian max opacity over the pixel box:
  strong (top 64 by max_a): exact l = ln(1-a), PACKED 2-up -> the ln
    pass covers two 512-px blocks per 128-partition tile (half width).
  nonstrong (bottom 64, max_a <= ~0.37): l ~= -(a + a^2/2); the
    quadratic-truncation error gives rel-err ~8e-4 (gate 2e-2).  The
    polynomial is ONE idle-DVE op: p' = (a+2)*a = 2a+a^2, with the 1/2
    folded into -0.5 entries of the triangular matmul weights.
ACT passes: exp1 (1.0) + ln (0.5) + exp2 (1.0) = 2.5 vs 3.0, and each
superstep is one [128,2048] PSUM tile so exp1/exp2 are single wide
instructions (fewer fixed ~240ns/instr overheads).

Layout per 2048-px superstep: one PSUM tile z [128, 2048] f32:
  cols    0:512  piece A: strong slots, blocks b0 (rows 0:64) | b1 (64:128)
  cols  512:1024 piece B: strong, b2 | b3
  cols 1024:1536 piece C: nonstrong, b0 | b1
  cols 1536:2048 piece D: nonstrong, b2 | b3
mm1: 4 block-diagonal matmuls (18-row error-compensated bf16 coeffs at
strip rows 0:18 / 32:50 of the v1 f18p feature layout).
ACT exp1: a = exp(z) [128,2048] f32.  ACT ln: l = ln(1-a[:,0:1024]) bf16.
DVE: p' = (a_ns+2)*a_ns bf16.  mm2: 8 matmuls accumulate S onto z
(blockdiag strict-order tri matrices; -0.5 weights for p').  ACT exp2:
w = exp(z+S) bf16.  mm3: 4 matmuls -> z[0:12,0:512] (freed rows), DVE
copy -> SBUF, DMA out.  exp2/mm3/copy are skewed one superstep so ACT
fills the mm2 latency with the next exp1.  Host adds bg and unpacks.
"""

import numpy as np
import ml_dtypes

import concourse.bass as bass
import concourse.bacc as bacc
import concourse.mybir as mybir
import concourse.tile as tile
from concourse.bass_utils import run_bass_kernel_spmd

G = 128
B = 4
N = 65536
BN = B * N
NCORES = 8
PPC = BN // NCORES          # pixels per core = 32768
SUPPX = 2048                # pixels per superstep
NSUP = PPC // SUPPX         # 16
FW = 1024                   # f18p cols per superstep (2 blocks packed/col)

F32 = mybir.dt.float32
BF16 = mybir.dt.bfloat16
AFT = mybir.ActivationFunctionType
ALU = mybir.AluOpType
BF = ml_dtypes.bfloat16

PROFILE = False
LAST_EXEC_NS = None
LAST_RESULTS = None

_cached = None


def _patch_act_tables():
    """Force every activation onto the one table set that has BOTH Exp and
    Ln ("natural_log_exp_and_others") — otherwise the table-load pass
    alternates sets and burns ~1.3us per ACT_TABLE_LOAD, once per tile."""
    if getattr(bacc, "_act_tables_patched", False):
        return
    orig = bacc.get_activation_tables

    def only_nle(arch):
        tabs = orig(arch)
        return {
            name: (fns if name == "natural_log_exp_and_others" else set())
            for name, fns in tabs.items()
        }

    bacc.get_activation_tables = only_nle
    bacc._act_tables_patched = True


def _build():
    _patch_act_tables()
    nc = bacc.Bacc("TRN2", target_bir_lowering=False, debug=False,
                   num_devices=NCORES)
    f18p = nc.dram_tensor("f18p", [64, PPC // 2], BF16, kind="ExternalInput")
    w1s = nc.dram_tensor("w1s", [64, G], BF16, kind="ExternalInput")
    w1n = nc.dram_tensor("w1n", [64, G], BF16, kind="ExternalInput")
    tss = nc.dram_tensor("tss", [G, G], BF16, kind="ExternalInput")
    tsn = nc.dram_tensor("tsn", [G, G], BF16, kind="ExternalInput")
    tns = nc.dram_tensor("tns", [G, G], BF16, kind="ExternalInput")
    tnn = nc.dram_tensor("tnn", [G, G], BF16, kind="ExternalInput")
    w3s = nc.dram_tensor("w3s", [G, 6], BF16, kind="ExternalInput")
    w3n = nc.dram_tensor("w3n", [G, 6], BF16, kind="ExternalInput")
    out = nc.dram_tensor("out", [6, PPC // 2], F32, kind="ExternalOutput")

    with tile.TileContext(nc) as tc:
        with (
            tc.tile_pool(name="const", bufs=1) as constp,
            tc.tile_pool(name="featp", bufs=3) as featp,
            tc.tile_pool(name="zsp", bufs=2, space="PSUM") as zsp,
            tc.tile_pool(name="znp", bufs=2, space="PSUM") as znp,
            tc.tile_pool(name="aa", bufs=3) as ap_,
            tc.tile_pool(name="lp", bufs=3) as lp,
            tc.tile_pool(name="pp", bufs=3) as pp,
            tc.tile_pool(name="wp", bufs=3) as wp,
            tc.tile_pool(name="obuf", bufs=4) as obufp,
        ):
            # dependency-free dummy activation: pulls the ~1.3us
            # ACT_TABLE_LOAD into the idle DMA-wait head
            dummy = constp.tile([1, 8], F32)
            nc.gpsimd.memset(dummy[:], 0.0)
            nc.scalar.activation(dummy[:], dummy[:], AFT.Exp)

            fbufs = [featp.tile([64, FW], BF16, tag="fbuf", name=f"fbuf{i}")
                     for i in range(NSUP)]
            # mm1 weights first (small), then the first feature block in
            # two chunks so the first matmul starts as early as possible;
            # later blocks ride the gpsimd queue
            w1s_t = constp.tile([64, G], BF16)
            nc.sync.dma_start(w1s_t[:], w1s[:])
            w1n_t = constp.tile([64, G], BF16)
            nc.sync.dma_start(w1n_t[:], w1n[:])
            nc.sync.dma_start(fbufs[0][:, 0:512], f18p[:, 0:512])
            nc.sync.dma_start(fbufs[0][:, 512:1024], f18p[:, 512:1024])
            nc.gpsimd.dma_start(fbufs[1][:], f18p[:, bass.ts(1, FW)])
            tss_t = constp.tile([G, G], BF16)
            nc.gpsimd.dma_start(tss_t[:], tss[:])
            tsn_t = constp.tile([G, G], BF16)
            nc.gpsimd.dma_start(tsn_t[:], tsn[:])
            tns_t = constp.tile([G, G], BF16)
            nc.gpsimd.dma_start(tns_t[:], tns[:])
            tnn_t = constp.tile([G, G], BF16)
            nc.gpsimd.dma_start(tnn_t[:], tnn[:])
            w3s_t = constp.tile([G, 6], BF16)
            nc.gpsimd.dma_start(w3s_t[:], w3s[:])
            w3n_t = constp.tile([G, 6], BF16)
            nc.gpsimd.dma_start(w3n_t[:], w3n[:])

            zss = [None] * NSUP
            zns = [None] * NSUP
            fbs = [None] * NSUP

            def mm1(p):
                fb = fbufs[p]
                zs = zsp.tile([G, 1024], F32, tag="zs", name=f"zs{p}")
                zn = znp.tile([G, 1024], F32, tag="zn", name=f"zn{p}")
                zss[p] = zs
                zns[p] = zn
                for h in range(2):
                    fph = fb[:, bass.ts(h, 512)]
                    dst = bass.ds(h * 512, 512)
                    nc.tensor.matmul(zs[:, dst], w1s_t[:], fph,
                                     start=True, stop=False)
                    nc.tensor.matmul(zn[:, dst], w1n_t[:], fph,
                                     start=True, stop=False)

            # prologue: features for supersteps 0/1 are already requested;
            # mm1(0) must precede the loop so exp1(0) has its input
            mm1(0)
            for p in range(NSUP + 1):
                if p >= 1:
                    q = p - 1
                    zs, zn = zss[q], zns[q]
                    w = wp.tile([G, 2048], BF16, tag="w", name=f"w{q}")
                    nc.scalar.activation(w[:, 0:1024], zs[:], AFT.Exp)
                    nc.scalar.activation(w[:, 1024:2048], zn[:], AFT.Exp)
                    if p < NSUP:
                        mm1(p + 1) if p + 1 < NSUP else None
                    for h in range(2):
                        dst = bass.ds(h * 512, 512)
                        nc.tensor.matmul(zn[0:6, dst], w3s_t[:],
                                         w[:, bass.ds(h * 512, 512)],
                                         start=True, stop=False)
                        nc.tensor.matmul(zn[0:6, dst], w3n_t[:],
                                         w[:, bass.ds(1024 + h * 512, 512)],
                                         start=False, stop=True)
                    ob = obufp.tile([6, 1024], F32, tag="ob", name=f"ob{q}")
                    nc.vector.tensor_copy(ob[:], zn[0:6, 0:1024])
                    nc.sync.dma_start(out[:, bass.ts(q, 1024)], ob[:])
                if p < NSUP:
                    if p + 2 < NSUP:
                        nc.gpsimd.dma_start(fbufs[p + 2][:],
                                            f18p[:, bass.ts(p + 2, FW)])
                    if p == 0 and NSUP > 1:
                        mm1(1)
                    zs, zn = zss[p], zns[p]
                    a = ap_.tile([G, 2048], F32, tag="a", name=f"a{p}")
                    nc.scalar.activation(a[:, 0:1024], zs[:], AFT.Exp)
                    nc.scalar.activation(a[:, 1024:2048], zn[:], AFT.Exp)
                    l = lp.tile([G, 1024], BF16, tag="l", name=f"l{p}")
                    nc.scalar.activation(l[:], a[:, 0:1024], AFT.Ln,
                                         bias=1.0, scale=-1.0)
                    pq = pp.tile([G, 1024], BF16, tag="pq", name=f"pq{p}")
                    nc.vector.scalar_tensor_tensor(
                        pq[:], a[:, 1024:2048], 2.0, a[:, 1024:2048],
                        ALU.add, ALU.mult)
                    for h in range(2):
                        sl = l[:, bass.ts(h, 512)]
                        sp = pq[:, bass.ts(h, 512)]
                        dst = bass.ds(h * 512, 512)
                        nc.tensor.matmul(zs[:, dst], tss_t[:], sl,
                                         start=False, stop=False)
                        nc.tensor.matmul(zs[:, dst], tns_t[:], sp,
                                         start=False, stop=True)
                        nc.tensor.matmul(zn[:, dst], tsn_t[:], sl,
                                         start=False, stop=False)
                        nc.tensor.matmul(zn[:, dst], tnn_t[:], sp,
                                         start=False, stop=True)

    nc.compile()
    return nc


def _box_max_z(coeffs):
    """Per-gaussian max of z(x) = k0 x0^2 + k1 x0 x1 + k2 x1^2 + k3 x0 +
    k4 x1 + k5 over the pixel box [-2,2]^2 (k0,k2 < 0: concave)."""
    k0, k1, k2, k3, k4, k5 = [coeffs[:, i] for i in range(6)]

    def ev(x0, x1):
        return (k0 * x0 * x0 + k1 * x0 * x1 + k2 * x1 * x1
                + k3 * x0 + k4 * x1 + k5)

    best = np.full(coeffs.shape[0], -np.inf)
    det = 4.0 * k0 * k2 - k1 * k1
    x0u = (-2.0 * k2 * k3 + k1 * k4) / det
    x1u = (-2.0 * k0 * k4 + k1 * k3) / det
    inside = (np.abs(x0u) <= 2.0) & (np.abs(x1u) <= 2.0) & (det > 0)
    best = np.where(inside, ev(np.clip(x0u, -2, 2), np.clip(x1u, -2, 2)), best)
    for s in (-2.0, 2.0):
        x1e = np.clip(-(k1 * s + k4) / (2.0 * k2), -2.0, 2.0)
        best = np.maximum(best, ev(s, x1e))
        x0e = np.clip(-(k1 * s + k3) / (2.0 * k0), -2.0, 2.0)
        best = np.maximum(best, ev(x0e, s))
    return best


def _host_prep(mu, chol, alpha, rgb, rotation, translation, projection, bg):
    # sort by camera distance in fp32 (matches reference argsort exactly)
    d32 = (mu.astype(np.float32) - translation.astype(np.float32)[None, :])
    dist = np.sqrt(np.sum(d32 * d32, axis=-1, dtype=np.float32))
    order = np.argsort(dist, kind="stable")
    mu = mu.astype(np.float64)[order]
    chol = chol.astype(np.float64)[order]
    alpha = alpha.astype(np.float64)[order]
    rgb = rgb.astype(np.float64)[order]
    rotation = rotation.astype(np.float64)
    translation = translation.astype(np.float64)
    projection = projection.astype(np.float64)
    bg = bg.astype(np.float64)

    inv_rot = rotation.T
    inv_trans = -inv_rot @ translation
    Lg = np.tril(chol) + 0.3 * np.eye(3)
    Sigma = np.einsum("gij,gkj->gik", Lg, Lg)
    mu_cam = np.einsum("ij,gj->gi", inv_rot, mu) + inv_trans
    mu2d = np.einsum("ij,gj->gi", projection, mu_cam)
    P_cam = projection @ inv_rot
    S2 = np.einsum("ij,gjk,lk->gil", P_cam, Sigma, P_cam) + 1e-4 * np.eye(2)
    det = S2[:, 0, 0] * S2[:, 1, 1] - S2[:, 0, 1] * S2[:, 1, 0]
    inv = np.empty((G, 2, 2))
    inv[:, 0, 0] = S2[:, 1, 1]
    inv[:, 0, 1] = -S2[:, 0, 1]
    inv[:, 1, 0] = -S2[:, 1, 0]
    inv[:, 1, 1] = S2[:, 0, 0]
    inv /= det[:, None, None]

    sp_ = np.logaddexp(0.0, alpha)
    wg = sp_ / (1.0 + sp_)
    color = rgb / (1.0 + np.abs(rgb))

    A = inv[:, 0, 0]
    Bc = inv[:, 0, 1] + inv[:, 1, 0]
    C = inv[:, 1, 1]
    m0, m1 = mu2d[:, 0], mu2d[:, 1]
    D = -2 * A * m0 - Bc * m1
    E = -Bc * m0 - 2 * C * m1
    F = A * m0 ** 2 + Bc * m0 * m1 + C * m1 ** 2
    coeffs = -0.5 * np.stack([A, Bc, C, D, E, F], axis=1)  # [G, 6]
    coeffs[:, 5] += np.log(wg)

    # strong = top 64 by max opacity over the pixel box; the rest get the
    # quadratic ln(1-a) approximation
    maxz = _box_max_z(coeffs)
    rk = np.argsort(-maxz, kind="stable")
    strong = np.sort(rk[:64])
    ns = np.sort(rk[64:])

    coefT = np.ascontiguousarray(coeffs.T).astype(np.float32)        # [6, G]
    ch = coefT.astype(BF)
    cl = (coefT - ch.astype(np.float32)).astype(BF)
    c18 = np.concatenate([ch, ch, cl], axis=0).astype(BF)            # [18, G]

    def mk_w1(sel):
        w = np.zeros((64, G), BF)
        w[0:18, 0:64] = c18[:, sel]
        w[32:50, 64:128] = c18[:, sel]
        return w

    def mk_tri(rows, cols, scale):
        m = (rows[:, None] > cols[None, :]).astype(np.float32) * scale
        t = np.zeros((G, G), np.float32)
        t[0:64, 0:64] = m
        t[64:128, 64:128] = m
        return t.astype(BF)

    def mk_w3(sel, colmb):
        w = np.zeros((G, 6), BF)
        w[0:64, 0:3] = colmb[sel]
        w[64:128, 3:6] = colmb[sel]
        return w

    colmb = (color - bg[None, :]).astype(BF)
    consts = {
        "w1s": mk_w1(strong),
        "w1n": mk_w1(ns),
        "tss": mk_tri(strong, strong, 1.0),
        "tsn": mk_tri(strong, ns, 1.0),
        "tns": mk_tri(ns, strong, -0.5),
        "tnn": mk_tri(ns, ns, -0.5),
        "w3s": mk_w3(strong, colmb),
        "w3n": mk_w3(ns, colmb),
    }
    return consts, bg.astype(np.float32)


def kernel(x, mu, chol, alpha, rgb, rotation, translation, projection,
           background_color):
    global _cached, LAST_EXEC_NS, LAST_RESULTS
    x = np.asarray(x, np.float32)
    consts, bg = _host_prep(
        np.asarray(mu), np.asarray(chol), np.asarray(alpha), np.asarray(rgb),
        np.asarray(rotation), np.asarray(translation), np.asarray(projection),
        np.asarray(background_color))

    xf = x.reshape(BN, 2).astype(np.float64)
    feat = np.empty((6, BN), np.float32)
    feat[0] = xf[:, 0] ** 2
    feat[1] = xf[:, 0] * xf[:, 1]
    feat[2] = xf[:, 1] ** 2
    feat[3] = xf[:, 0]
    feat[4] = xf[:, 1]
    feat[5] = 1.0
    fh = feat.astype(BF)
    fl = (feat - fh.astype(np.float32)).astype(BF)
    f18 = np.concatenate([fh, fl, fh], axis=0)                       # [18, BN]

    if _cached is None:
        _cached = _build()
    nc = _cached

    in_maps = []
    for k in range(NCORES):
        fc = f18[:, k * PPC:(k + 1) * PPC].reshape(18, PPC // 512, 512)
        f18p = np.zeros((64, PPC // 2), BF)
        f18p[0:18] = fc[:, 0::2].reshape(18, PPC // 2)
        f18p[32:50] = fc[:, 1::2].reshape(18, PPC // 2)
        m = {"f18p": f18p}
        m.update(consts)
        in_maps.append(m)

    kwargs = {}
    if PROFILE:
        kwargs = dict(trace=True)
    res = run_bass_kernel_spmd(nc, in_maps, core_ids=list(range(NCORES)),
                               **kwargs)
    LAST_EXEC_NS = res.exec_time_ns
    LAST_RESULTS = res
    # out [6, PPC/2]: row r = q2*3 + c (q2: even/odd 512-block); col =
    # p*1024 + h*512 + i  ->  pixel = 2048p + (2h + q2)*512 + i
    full = np.empty((BN, 3), np.float32)
    for k in range(NCORES):
        o = res.results[k]["out"].reshape(2, 3, NSUP, 2, 512)
        full[k * PPC:(k + 1) * PPC] = (
            np.transpose(o, (2, 3, 0, 4, 1)).reshape(PPC, 3))
    return (full + bg[None, :]).reshape(B, N, 3).astype(np.float32)
